# revision 16
# baseline (speedup 1.0000x reference)
"""Gaussian-noise kernel for Trainium2: out = clip(x + noise, 0, 1).

Full input shape (64, 3, 512, 512) f32; pure data-parallel over the batch
dim across 8 NeuronCores (8 images per core). Per core the work is a flat
elementwise pass over 6,291,456 floats: DMA x and noise tiles into SBUF,
add, clip with one dual-op tensor_scalar (max 0, min 1), DMA the result
back out.

The per-core flat buffer is viewed as [N_CHUNKS, 128, CHUNK] so each
chunk's DMA is one fully contiguous block of DRAM.  Engine assignment per
stream is configurable: "sync" (SP HWDGE ring), "scalar" (ACT HWDGE ring),
"gpsimd" (SWDGE ring).  With accum=True the noise load is a SWDGE DMA with
accum_op=add that adds into the x tile in the DMA datapath, so the vector
engine only does the clip.
"""

import numpy as np

import concourse.bacc as bacc
import concourse.bass as bass
import concourse.mybir as mybir
from concourse.bass_utils import run_bass_kernel_spmd
from concourse.tile import TileContext

N_CORES = 8
B, C, H, W = 64, 3, 512, 512
PER_CORE_ELEMS = (B // N_CORES) * C * H * W  # 6,291,456
P = 128
FREE = PER_CORE_ELEMS // P  # 49,152

# tuned knobs — the config kernel() runs with and test.py benches.
# Segmented read/write phases: per pass, 2 segments of [24 MiB pure-read
# burst (loads + compute into 12 held tiles)][12 MiB pure-write burst].
# Mixed-direction HBM traffic measures ~5-8% slower than single-direction
# bursts, so phase separation beats every interleaved schedule.
BUILD_KWARGS = dict(
    chunk=2048,
    seg=12,
    n_bufs=3,
    x_eng=("sync", "scalar"),
    n_eng=("scalar", "sync"),
    s_eng=("sync", "scalar"),
)
CHUNK = BUILD_KWARGS["chunk"]

_cached_nc = None


def _engine(nc, name):
    return {"sync": nc.sync, "scalar": nc.scalar, "gpsimd": nc.gpsimd}[name]


def _pick(spec, i):
    """spec is an engine name or tuple of names cycled by chunk index."""
    if isinstance(spec, (tuple, list)):
        return spec[i % len(spec)]
    return spec


def _build(repeat: int = 1, chunk: int = 4096, bufs: int = 3,
           x_eng="sync", n_eng="scalar", s_eng="scalar",
           accum: bool = False, taper: bool = False, store_lag: int = 0,
           probe=None, park: bool = False, n_bufs: int = 2, seg: int = 0,
           n_half: bool = False):
    n_chunks = FREE // chunk
    assert n_chunks * chunk == FREE
    assert store_lag < n_chunks

    nc = bacc.Bacc("TRN2", target_bir_lowering=False, debug=False)
    f32 = mybir.dt.float32
    shape = (n_chunks, P, chunk)
    x = nc.dram_tensor("x", shape, f32, kind="ExternalInput").ap()
    noise = nc.dram_tensor("noise", shape, f32, kind="ExternalInput").ap()
    out = nc.dram_tensor("out", shape, f32, kind="ExternalOutput").ap()

    with TileContext(nc) as tc:
        with tc.tile_pool(name="io", bufs=bufs) as pool:

            def emit_front(i, lo, width):
                """Loads + compute for chunk i; returns the result tile."""
                sub = (lambda ap: ap[i] if width == chunk
                       else ap[i][:, lo:lo + width])
                xt = pool.tile([P, width], f32, tag="x")
                _engine(nc, _pick(x_eng, i)).dma_start(out=xt, in_=sub(x))
                if accum:
                    nc.gpsimd.dma_start(out=xt, in_=sub(noise),
                                        accum_op=mybir.AluOpType.add)
                else:
                    nt = pool.tile([P, width], f32, tag="n")
                    _engine(nc, _pick(n_eng, i)).dma_start(out=nt, in_=sub(noise))
                    nc.vector.tensor_add(out=xt, in0=xt, in1=nt)
                nc.vector.tensor_scalar(
                    out=xt,
                    in0=xt,
                    scalar1=0.0,
                    scalar2=1.0,
                    op0=mybir.AluOpType.max,
                    op1=mybir.AluOpType.min,
                )
                return xt

            def emit_store(i, xt, lo, width):
                sub = (lambda ap: ap[i] if width == chunk
                       else ap[i][:, lo:lo + width])
                _engine(nc, _pick(s_eng, i)).dma_start(out=sub(out), in_=xt)

            def body_probe():
                """Bandwidth probes: loads only, or stores only."""
                for i in range(n_chunks):
                    if probe == "loadonly":
                        xt = pool.tile([P, chunk], f32, tag="x")
                        nt = pool.tile([P, chunk], f32, tag="n")
                        _engine(nc, _pick(x_eng, i)).dma_start(out=xt, in_=x[i])
                        _engine(nc, _pick(n_eng, i)).dma_start(out=nt, in_=noise[i])
                    elif probe == "storeonly":
                        xt = pool.tile([P, chunk], f32, tag="x")
                        nc.vector.memset(xt, 0.25)
                        _engine(nc, _pick(s_eng, i)).dma_start(out=out[i], in_=xt)
                    elif probe == "mixed":
                        # loads and stores with no data dependency between them
                        xt = pool.tile([P, chunk], f32, tag="x")
                        nt = pool.tile([P, chunk], f32, tag="n")
                        st = pool.tile([P, chunk], f32, tag="s")
                        _engine(nc, _pick(x_eng, i)).dma_start(out=xt, in_=x[i])
                        _engine(nc, _pick(n_eng, i)).dma_start(out=nt, in_=noise[i])
                        nc.vector.memset(st, 0.25)
                        _engine(nc, _pick(s_eng, i)).dma_start(out=out[i], in_=st)
                    else:
                        raise ValueError(probe)

            def body_park():
                """Phase-separated pass: pure-read phase computes into parked
                SBUF tiles; pure-write phase stores them.  Minimizes HBM
                read/write interleaving (mixed traffic measures ~5% slower
                than the serial sum of pure phases)."""
                parked = []
                for i in range(n_chunks):
                    xt = pool.tile([P, chunk], f32, tag="x", bufs=n_chunks)
                    nt = pool.tile([P, chunk], f32, tag="n", bufs=n_bufs)
                    _engine(nc, _pick(x_eng, i)).dma_start(out=xt, in_=x[i])
                    _engine(nc, _pick(n_eng, i)).dma_start(out=nt, in_=noise[i])
                    nc.vector.tensor_add(out=xt, in0=xt, in1=nt)
                    nc.vector.tensor_scalar(
                        out=xt, in0=xt, scalar1=0.0, scalar2=1.0,
                        op0=mybir.AluOpType.max, op1=mybir.AluOpType.min,
                    )
                    parked.append(xt)
                for i, xt in enumerate(parked):
                    _engine(nc, _pick(s_eng, i)).dma_start(out=out[i], in_=xt)

            def body_seg():
                """Segmented phases: S chunks of pure reads (+compute into
                held tiles), then S stores as a pure-write burst.  Each
                engine's FIFO orders loads(k) < stores(k) < loads(k+1), so
                the HBM sees long single-direction bursts instead of
                packet-interleaved read/write traffic."""
                for s0 in range(0, n_chunks, seg):
                    hi = min(s0 + seg, n_chunks)
                    held = []
                    for i in range(s0, hi):
                        xt = pool.tile([P, chunk], f32, tag="x",
                                       bufs=min(seg + 1, n_chunks))
                        _engine(nc, _pick(x_eng, i)).dma_start(out=xt, in_=x[i])
                        if n_half:
                            h2 = chunk // 2
                            for h in range(2):
                                nt = pool.tile([P, h2], f32, tag="n",
                                               bufs=n_bufs)
                                _engine(nc, _pick(n_eng, 2 * i + h)).dma_start(
                                    out=nt, in_=noise[i][:, h * h2:(h + 1) * h2])
                                nc.vector.tensor_add(
                                    out=xt[:, h * h2:(h + 1) * h2],
                                    in0=xt[:, h * h2:(h + 1) * h2], in1=nt)
                        else:
                            nt = pool.tile([P, chunk], f32, tag="n", bufs=n_bufs)
                            _engine(nc, _pick(n_eng, i)).dma_start(out=nt,
                                                                   in_=noise[i])
                            nc.vector.tensor_add(out=xt, in0=xt, in1=nt)
                        nc.vector.tensor_scalar(
                            out=xt, in0=xt, scalar1=0.0, scalar2=1.0,
                            op0=mybir.AluOpType.max, op1=mybir.AluOpType.min,
                        )
                        held.append((i, xt))
                    for i, xt in held:
                        _engine(nc, _pick(s_eng, i)).dma_start(out=out[i], in_=xt)

            def body():
                if seg:
                    body_seg()
                    return
                if park:
                    body_park()
                    return
                if probe:
                    body_probe()
                    return
                pending = []  # (chunk index, result tile, lo, width)

                def push(i, lo, width):
                    pending.append((i, emit_front(i, lo, width), lo, width))
                    if len(pending) > store_lag:
                        emit_store(*pending.pop(0))

                for i in range(n_chunks):
                    if taper and i in (0, n_chunks - 1):
                        half = chunk // 2
                        push(i, 0, half)
                        push(i, half, half)
                    else:
                        push(i, 0, chunk)
                while pending:
                    emit_store(*pending.pop(0))

            if repeat == 1:
                body()
            else:
                with tc.For_i(0, repeat, 1):
                    body()
    nc.compile()
    return nc


def _get_nc():
    global _cached_nc
    if _cached_nc is None:
        _cached_nc = _build(**BUILD_KWARGS)
    return _cached_nc


def _shard(a: np.ndarray, chunk: int = CHUNK):
    n_chunks = FREE // chunk
    a = np.ascontiguousarray(a, dtype=np.float32)
    return a.reshape(N_CORES, n_chunks, P, chunk)


# Cached PJRT executor: trace/compile the sharded bass_exec once per process
# so repeat kernel() calls only pay data transfer + execution.
_cached_fn = None


def _get_fn():
    global _cached_fn
    if _cached_fn is not None:
        return _cached_fn

    import jax
    from jax.sharding import Mesh, NamedSharding, PartitionSpec
    from jax.experimental.shard_map import shard_map
    from concourse.bass2jax import (
        _bass_exec_p,
        install_neuronx_cc_hook,
        partition_id_tensor,
    )

    nc = _get_nc()
    install_neuronx_cc_hook()
    partition_name = nc.partition_id_tensor.name if nc.partition_id_tensor else None

    in_names, out_names, out_avals, zero_outs = [], [], [], []
    for alloc in nc.m.functions[0].allocations:
        if not isinstance(alloc, mybir.MemoryLocationSet):
            continue
        name = alloc.memorylocations[0].name
        if alloc.kind == "ExternalInput":
            if name != partition_name:
                in_names.append(name)
        elif alloc.kind == "ExternalOutput":
            out_names.append(name)
            shape = tuple(alloc.tensor_shape)
            dtype = mybir.dt.np(alloc.dtype)
            out_avals.append(jax.core.ShapedArray(shape, dtype))
            zero_outs.append(np.zeros(shape, dtype))
    n_params = len(in_names)
    all_in_names = list(in_names) + list(out_names)
    if partition_name is not None:
        all_in_names.append(partition_name)

    def _body(*args):
        operands = list(args)
        if partition_name is not None:
            operands.append(partition_id_tensor())
        outs = _bass_exec_p.bind(
            *operands,
            out_avals=tuple(out_avals),
            in_names=tuple(all_in_names),
            out_names=tuple(out_names),
            lowering_input_output_aliases=(),
            sim_require_finite=True,
            sim_require_nnan=True,
            nc=nc,
        )
        return tuple(outs)

    devices = jax.devices()[:N_CORES]
    mesh = Mesh(np.asarray(devices), ("core",))
    in_specs = (PartitionSpec("core"),) * (n_params + len(out_names))
    out_specs = (PartitionSpec("core"),) * len(out_names)
    fn = jax.jit(
        shard_map(_body, mesh=mesh, in_specs=in_specs, out_specs=out_specs,
                  check_rep=False),
        keep_unused=True,
    )
    sharding = NamedSharding(mesh, PartitionSpec("core"))
    zeros_global = [np.concatenate([z] * N_CORES, axis=0) for z in zero_outs]
    _cached_fn = (fn, in_names, sharding, zeros_global)
    return _cached_fn


def _kernel_fast(x: np.ndarray, noise: np.ndarray) -> np.ndarray:
    import jax

    fn, in_names, sharding, zeros_global = _get_fn()
    per_core = {"x": _shard(x), "noise": _shard(noise)}
    args = []
    for name in in_names:
        a = per_core[name]
        args.append(jax.device_put(
            np.ascontiguousarray(a.reshape(-1, *a.shape[2:])), sharding))
    for z in zeros_global:
        args.append(jax.device_put(z, sharding))
    out = fn(*args)[0]
    return np.asarray(out).reshape(B, C, H, W)


def _kernel_stock(x: np.ndarray, noise: np.ndarray) -> np.ndarray:
    nc = _get_nc()
    xs = _shard(x)
    ns = _shard(noise)
    in_maps = [{"x": xs[c], "noise": ns[c]} for c in range(N_CORES)]
    res = run_bass_kernel_spmd(nc, in_maps, core_ids=list(range(N_CORES)))
    out = np.stack([res.results[c]["out"] for c in range(N_CORES)])
    return out.reshape(B, C, H, W)


_fast_broken = False


def kernel(x: np.ndarray, noise: np.ndarray) -> np.ndarray:
    global _fast_broken
    if not _fast_broken:
        try:
            return _kernel_fast(x, noise)
        except Exception:
            _fast_broken = True
    return _kernel_stock(x, noise)
